# revision 29
# baseline (speedup 1.0000x reference)
"""Multi-head self-attention (RoPE, causal) Trainium2 Bass kernel.

Sharding: 8 cores = 4 batches x 2 head-groups (8 heads each).
Per core, for its batch b and head-group g:
    q/k = x_b @ W*[:, g] (pi-permuted cols), RoPE, causal softmax
    attention, partial out-projection y @ Wo[g] -> [2048, 1024] bf16.
Host sums the two head-group partials per batch (f32) and adds bo and
the exact bv correction (softmax rows sum to 1, so the v bias passes
through attention unchanged -> + bv @ Wo on host).  q/k biases are
zero per the problem spec (asserted).

Device-side structure (v2, V-stationary attention):
    qTf/kTf [128, 4, 2048] bf16: head-pair packed (even head dims on
        partitions 0-63, odd on 64-127).  QK matmuls contract over 64
        partitions (base_partition 0 or 64) - no zero padding.
    att [128, 17408] bf16 per head (3 bufs across 2 pools): exp-scores,
        causal trapezoid packed, [k-part, q-free].
    AV: stationary = 128-col [V_e|ones|zeros|V_o] blocks from v_pair,
        moving = att columns -> psum [d-part, q-free] = yT directly
        (no transpose stage); softmax denominator row falls out of the
        ones column at partition 64 (even heads) / 0 (odd heads).
    normalize: reciprocal_approx_fast on the denom row, bf16 cast,
        partition-broadcast DMA to [64, 2048], one DVE mul per head.
    out-projection: yT-stationary, accumulate 4 head-pair blocks,
        bf16 output.
"""

import os
import sys

import numpy as np

for _p in ("/opt/trn_rl_repo", "/root/.axon_site/_ro/trn_rl_repo"):
    if os.path.isdir(_p) and _p not in sys.path:
        sys.path.append(_p)

import ml_dtypes  # noqa: E402

BF16 = ml_dtypes.bfloat16

B, S, D_MODEL = 4, 2048, 1024
N_HEADS, HEAD_DIM = 16, 64
N_CORES = 8
HG = 2                      # head groups
HPC = N_HEADS // HG         # heads per core = 8
DL = HPC * HEAD_DIM         # local dims per core = 512
SCALE = HEAD_DIM ** -0.5
P = 128
KC = D_MODEL // P           # k chunks in projections = 8
MB = DL // P                # head pairs per core = 4
NKB = S // P                # 128-row blocks of sequence = 16
NCH = 4                     # 512-wide q chunks per AV pass
CHW = S // NCH              # 512
RW = 1024                   # RoPE working-chunk width
EXPW = 1536                 # exp chunk width (3 psum banks)
VPW = 193                   # v_pair pair cols: V_e(64)|ones|zeros(63)|V_o(64)

# packed causal-trapezoid offsets: att row-block ck covers q in [128*ck, S)
ATT_OFF = [0] * (NKB + 1)
for _ck in range(NKB):
    ATT_OFF[_ck + 1] = ATT_OFF[_ck] + (S - P * _ck)
ATT_TOT = ATT_OFF[NKB]      # 17408

_CACHE = {}
DEBUG_TAPS = False


def _build_bass():
    import concourse.tile as tile
    from concourse import bacc, mybir

    dt = mybir.dt
    nc = bacc.Bacc("TRN2", target_bir_lowering=False, debug=False)

    def din(name, shape, d=dt.bfloat16):
        return nc.dram_tensor(name, shape, d, kind="ExternalInput").ap()

    xT_d = din("xT", [D_MODEL, S])
    wq_d = din("wq", [D_MODEL, DL])
    wk_d = din("wk", [D_MODEL, DL])
    wv_d = din("wv", [D_MODEL, DL])
    wo_d = din("wo", [DL, D_MODEL])
    cos_d = din("cosT", [P, S])
    sin_d = din("sinT", [P, S])          # sign-folded (pi-basis)
    ident_d = din("ident", [P, P])
    mneg_d = din("mneg", [P, P])         # -240 strictly-lower tri [k, q]
    o_d = nc.dram_tensor("o", [S, D_MODEL], dt.bfloat16,
                         kind="ExternalOutput").ap()
    # DRAM scratch for the softmax denominator/reciprocal rows: SBUF
    # APs can neither refold partition dims nor partition-broadcast,
    # so both hops bounce through DRAM.
    rscrf_d = nc.dram_tensor("rscrf", [HPC, S], dt.float32,
                             kind="Internal").ap()
    rscrb_d = nc.dram_tensor("rscrb", [HPC, S], dt.bfloat16,
                             kind="Internal").ap()

    FCopy = mybir.ActivationFunctionType.Copy
    FExp = mybir.ActivationFunctionType.Exp

    with tile.TileContext(nc) as tc:
        with (
            tc.tile_pool(name="persist", bufs=1) as persist,
            tc.tile_pool(name="small", bufs=1) as small,
            tc.tile_pool(name="attE", bufs=2) as attE,
            tc.tile_pool(name="psA", bufs=2, space="PSUM") as psA,
            tc.tile_pool(name="psB", bufs=2, space="PSUM") as psB,
        ):
            qTf = persist.tile([P, MB, S], dt.bfloat16, tag="qTf")
            kTf = persist.tile([P, MB, S], dt.bfloat16, tag="kTf")
            v_pair = persist.tile([P, NKB, MB, VPW], dt.bfloat16, tag="vp")
            yT = persist.tile([P, MB, S], dt.bfloat16, tag="yT")
            ident_sb = small.tile([P, P], dt.bfloat16, tag="ident")
            mneg_sb = small.tile([P, P], dt.bfloat16, tag="mneg")

            # ones + zeros padding columns of v_pair (cols 64..127)
            nc.vector.memset(v_pair[:, :, :, HEAD_DIM:2 * HEAD_DIM], 0.0)
            nc.vector.memset(v_pair[:, :, :, HEAD_DIM:HEAD_DIM + 1], 1.0)

            att_tiles = {}

            # ---------------- attention emission helpers ----------------
            def emit_c1_steps(h, att):
                # QK + exp (+ causal mask on diag block); yields once per
                # psum chunk so the driver can interleave PE filler work.
                m, side = h // 2, h % 2
                sp = side * HEAD_DIM
                kh = kTf[sp:sp + HEAD_DIM, m, :]
                qh = qTf[sp:sp + HEAD_DIM, m, :]
                for ck in range(NKB):
                    w = S - ck * P
                    base = ck * P
                    off = ATT_OFF[ck]
                    for s0 in range(0, w, EXPW):
                        cw = min(EXPW, w - s0)
                        ps = psA.tile([P, EXPW], dt.float32, tag="qk")
                        if s0 == 0:
                            # causal mask: accumulate -240 onto the
                            # strictly-lower half of the diagonal block
                            # before the scores (exp maps it to ~0)
                            nc.tensor.matmul(
                                ps[:, 0:P], lhsT=ident_sb, rhs=mneg_sb,
                                start=True, stop=False)
                        for u0 in range(0, cw, CHW):
                            uw = min(CHW, cw - u0)
                            nc.tensor.matmul(
                                ps[:, u0:u0 + uw],
                                lhsT=kh[:, ck * P:(ck + 1) * P],
                                rhs=qh[:, base + s0 + u0:base + s0 + u0 + uw],
                                start=not (s0 == 0 and u0 == 0), stop=True)
                        nc.scalar.activation(
                            out=att[:, off + s0:off + s0 + cw],
                            in_=ps[:, 0:cw], func=FExp, scale=SCALE)
                        nc.tensor.ldweights(weights=ident_sb[:, 0:HEAD_DIM])
                        yield

            def make_c2(den_s, dng, dngb, rbc_pool):
                def emit_c2_steps(h, att):
                    # V-stationary AV per 512-q chunk; denominator from
                    # the ones column; yields every ~2 matmuls.
                    m, side = h // 2, h % 2
                    sp = side * HEAD_DIM
                    drow = HEAD_DIM if side == 0 else 0
                    nmm = 0
                    for j in range(NCH):
                        ps = psB.tile([P, CHW], dt.float32, tag="ps512")
                        last_ck = min(4 * j + 3, NKB - 1)
                        for ck in range(last_ck + 1):
                            d = ck - 4 * j
                            if d <= 0:
                                a0 = ATT_OFF[ck] + j * CHW - ck * P
                                c0, wd = 0, CHW
                            else:
                                a0 = ATT_OFF[ck]
                                c0, wd = d * P, CHW - d * P
                            nc.tensor.matmul(
                                ps[:, c0:c0 + wd],
                                lhsT=v_pair[:, ck, m, sp:sp + P],
                                rhs=att[:, a0:a0 + wd],
                                start=(ck == 0), stop=(ck == last_ck))
                            nmm += 1
                            if nmm % 2 == 0:
                                yield
                        js = slice(j * CHW, (j + 1) * CHW)
                        # raw (unnormalized) yT chunk -> SBUF
                        nc.vector.tensor_copy(
                            out=yT[sp:sp + HEAD_DIM, m, js],
                            in_=ps[sp:sp + HEAD_DIM, :])
                        # denominator chunk, staged per parity row (fp32)
                        nc.vector.tensor_copy(
                            out=den_s[drow:drow + 1, js],
                            in_=ps[drow:drow + 1, :])
                        yield
                    # head tail: den row -> DRAM -> [8, 256] at
                    # partition 0 (reciprocal_approx_fast only works at
                    # base partition 0) -> 1/x -> bf16 -> broadcast
                    nc.sync.dma_start(out=rscrf_d[h:h + 1, :],
                                      in_=den_s[drow:drow + 1, :])
                    nc.sync.dma_start(
                        out=dng,
                        in_=rscrf_d[h:h + 1, :].rearrange(
                            "a (p f) -> (a p) f", p=8))
                    nc.vector.reciprocal_approx_fast(out=dng, in_=dng)
                    nc.vector.tensor_copy(out=dngb, in_=dng)
                    nc.sync.dma_start(
                        out=rscrb_d[h:h + 1, :].rearrange(
                            "a (p f) -> (a p) f", p=8),
                        in_=dngb)
                    rbc = rbc_pool.tile([P, S], dt.bfloat16, tag="rbc")
                    nc.sync.dma_start(
                        out=rbc[sp:sp + HEAD_DIM, :],
                        in_=rscrb_d[h:h + 1, :].to_broadcast(
                            (HEAD_DIM, S)))
                    nc.vector.tensor_mul(
                        yT[sp:sp + HEAD_DIM, m, :],
                        yT[sp:sp + HEAD_DIM, m, :],
                        rbc[sp:sp + HEAD_DIM, :])
                    yield
                return emit_c2_steps

            def drive(c1, fillers, per=1):
                # emit c1 chunks, pulling filler steps per chunk
                for _ in c1:
                    for _ in range(per):
                        next(fillers, None)
                for _ in fillers:
                    pass

            # ======= Stage B: q/k/v projections + RoPE + h0/h1 c1 =======
            with tc.tile_pool(name="xtp", bufs=1) as xtp:
                xT_sb = xtp.tile([P, KC, S], dt.bfloat16, tag="xT")

                with (
                    tc.tile_pool(name="ropew", bufs=1) as ropew,
                    tc.tile_pool(name="bstage", bufs=2) as bstage,
                ):
                    wq_sb = ropew.tile([P, KC, DL], dt.bfloat16, tag="wq")
                    wk_sb = ropew.tile([P, KC, DL], dt.bfloat16, tag="wk")
                    cos_sb = ropew.tile([P, S], dt.bfloat16, tag="cos")
                    sin_sb = ropew.tile([P, S], dt.bfloat16, tag="sin")

                    # DMA order paces PE consumption: wq, first xT
                    # chunk, wk, then the rest
                    nc.sync.dma_start(
                        out=wq_sb,
                        in_=wq_d.rearrange("(kc p) n -> p kc n", p=P))
                    for t in range(NCH):
                        ts = slice(t * CHW, (t + 1) * CHW)
                        nc.sync.dma_start(
                            out=xT_sb[:, :, ts],
                            in_=xT_d[:, ts].rearrange(
                                "(kc p) s -> p kc s", p=P))
                        if t == 0:
                            nc.sync.dma_start(
                                out=wk_sb,
                                in_=wk_d.rearrange("(kc p) n -> p kc n",
                                                   p=P))
                        if t == 1:
                            nc.sync.dma_start(out=cos_sb, in_=cos_d)
                            nc.sync.dma_start(out=sin_sb, in_=sin_d)
                    nc.sync.dma_start(out=ident_sb, in_=ident_d)
                    nc.sync.dma_start(out=mneg_sb, in_=mneg_d)

                    raws = {}

                    def emit_proj_u(nm, m, rc, u):
                        # one 512-col projection chain into raw staging
                        w_sb = wq_sb if nm == "q" else wk_sb
                        if u == 0:
                            raws[nm] = bstage.tile([P, RW], dt.bfloat16,
                                                   tag="raw" + nm,
                                                   name="raw" + nm)
                        ts = slice(rc * RW + u * CHW,
                                   rc * RW + (u + 1) * CHW)
                        ps = psB.tile([P, CHW], dt.float32, tag="ps512")
                        for kc in range(KC):
                            nc.tensor.matmul(
                                ps, lhsT=w_sb[:, kc, m * P:(m + 1) * P],
                                rhs=xT_sb[:, kc, ts],
                                start=(kc == 0), stop=(kc == KC - 1))
                        nc.vector.tensor_copy(
                            out=raws[nm][:, u * CHW:(u + 1) * CHW],
                            in_=ps)

                    def emit_rope(nm, m, rc):
                        # rotate-half in the permuted basis: swap the
                        # 32-row halves per head (sign folded in sinT)
                        dst = qTf if nm == "q" else kTf
                        dma_eng = nc.scalar if nm == "q" else nc.sync
                        HH = HEAD_DIM // 2
                        rs = slice(rc * RW, (rc + 1) * RW)
                        raw = raws[nm]
                        t2t = bstage.tile([P, RW], dt.bfloat16,
                                          tag="t2t" + nm)
                        for hh in range(2):
                            o32 = hh * HEAD_DIM
                            dma_eng.dma_start(
                                out=t2t[o32:o32 + HH, :],
                                in_=raw[o32 + HH:o32 + HEAD_DIM, :])
                            dma_eng.dma_start(
                                out=t2t[o32 + HH:o32 + HEAD_DIM, :],
                                in_=raw[o32:o32 + HH, :])
                        nc.gpsimd.tensor_mul(t2t, t2t, sin_sb[:, rs])
                        nc.vector.tensor_mul(dst[:, m, rs], raw,
                                             cos_sb[:, rs])
                        nc.vector.tensor_add(dst[:, m, rs],
                                             dst[:, m, rs], t2t)

                    def emit_qk_proj(nm, m, rc):
                        for u in range(RW // CHW):
                            emit_proj_u(nm, m, rc, u)
                        emit_rope(nm, m, rc)

                    # first pair paced per-u against the input DMAs
                    for u in range(RW // CHW):
                        emit_proj_u("q", 0, 0, u)
                        emit_proj_u("k", 0, 0, u)
                    emit_rope("q", 0, 0)
                    emit_rope("k", 0, 0)
                    for m in range(1, MB):
                        emit_qk_proj("q", m, 0)
                        emit_qk_proj("k", m, 0)
                    emit_qk_proj("q", 0, 1)
                    emit_qk_proj("k", 0, 1)

                    def gen_qk_proj_rest():
                        for m in range(1, MB):
                            emit_qk_proj("q", m, 1)
                            yield
                            emit_qk_proj("k", m, 1)
                            yield

                    # h0 c1 interleaved with remaining q/k projections
                    att_tiles[0] = attE.tile([P, ATT_TOT], dt.bfloat16,
                                             tag="att", name="att0")
                    drive(emit_c1_steps(0, att_tiles[0]),
                          gen_qk_proj_rest())

                # ropew/bstage freed; v projection + h1 c1
                with tc.tile_pool(name="vw", bufs=1) as vw:
                    wv_sb = vw.tile([P, KC, DL], dt.bfloat16, tag="wv")
                    nc.sync.dma_start(
                        out=wv_sb,
                        in_=wv_d.rearrange("(kc p) n -> p kc n", p=P))

                    def gen_v_proj():
                        for kb in range(NKB):
                            ps = psB.tile([P, DL], dt.float32, tag="ps512")
                            for kc in range(KC):
                                nc.tensor.matmul(
                                    ps,
                                    lhsT=xT_sb[:, kc, kb * P:(kb + 1) * P],
                                    rhs=wv_sb[:, kc, :],
                                    start=(kc == 0), stop=(kc == KC - 1))
                                if kc % 4 == 3:
                                    yield
                            # scatter the 8 heads into even/odd slots
                            for sd in range(2):
                                nc.vector.tensor_copy(
                                    out=v_pair[:, kb, :,
                                               sd * 2 * HEAD_DIM:
                                               sd * 2 * HEAD_DIM + HEAD_DIM],
                                    in_=ps.rearrange(
                                        "p (m s d) -> p m s d",
                                        m=MB, s=2)[:, :, sd, :])
                            yield

                    att_tiles[1] = attE.tile([P, ATT_TOT], dt.bfloat16,
                                             tag="att", name="att1")
                    drive(emit_c1_steps(1, att_tiles[1]), gen_v_proj(), per=2)

            # ========= Stage C steady state + Stage D out-proj =========
            with (
                tc.tile_pool(name="late", bufs=1) as late,
                tc.tile_pool(name="attL", bufs=1) as attL,
                tc.tile_pool(name="rbcp", bufs=2) as rbc_pool,
                tc.tile_pool(name="osb", bufs=3) as osb,
            ):
                wo_sb = late.tile([P, MB, D_MODEL], dt.bfloat16, tag="wo")
                den_s = late.tile([P, S], dt.float32, tag="den_s")
                dng = late.tile([8, S // 8], dt.float32, tag="dng")
                dngb = late.tile([8, S // 8], dt.bfloat16, tag="dngb")
                nc.sync.dma_start(
                    out=wo_sb, in_=wo_d.rearrange("(m p) n -> p m n", p=P))

                emit_c2_steps = make_c2(den_s, dng, dngb, rbc_pool)

                for h in range(2, HPC):
                    pool = attL if h % 3 == 2 else attE
                    att_tiles[h] = pool.tile(
                        [P, ATT_TOT], dt.bfloat16, tag="att", name=f"att{h}")
                    drive(emit_c1_steps(h, att_tiles[h]),
                          emit_c2_steps(h - 2, att_tiles[h - 2]))
                for hh in (HPC - 2, HPC - 1):
                    for _ in emit_c2_steps(hh, att_tiles[hh]):
                        pass

                for qb in range(NKB):
                    ob = osb.tile([P, D_MODEL], dt.bfloat16, tag="ob")
                    for t in range(2):
                        ps = psB.tile([P, DL], dt.float32, tag="ps512")
                        for m in range(MB):
                            nc.tensor.matmul(
                                ps, lhsT=yT[:, m, qb * P:(qb + 1) * P],
                                rhs=wo_sb[:, m, t * DL:(t + 1) * DL],
                                start=(m == 0), stop=(m == MB - 1))
                        nc.vector.tensor_copy(
                            out=ob[:, t * DL:(t + 1) * DL], in_=ps)
                    nc.scalar.dma_start(
                        out=o_d[qb * P:(qb + 1) * P, :], in_=ob)

                if DEBUG_TAPS:
                    taps = {
                        "qTf_o": (qTf, [P, MB, S], dt.bfloat16),
                        "kTf_o": (kTf, [P, MB, S], dt.bfloat16),
                        "vp_o": (v_pair, [P, NKB, MB, VPW], dt.bfloat16),
                        "yT_o": (yT, [P, MB, S], dt.bfloat16),
                        "den_o": (den_s, [P, S], dt.float32),
                        "att6_o": (att_tiles[6], [P, ATT_TOT], dt.bfloat16),
                        "att7_o": (att_tiles[7], [P, ATT_TOT], dt.bfloat16),
                    }
                    for nm, (t_, shp, dd) in taps.items():
                        td = nc.dram_tensor(nm, shp, dd,
                                            kind="ExternalOutput").ap()
                        nc.sync.dma_start(out=td, in_=t_)

    nc.compile()
    return nc


def _perm64():
    # pi: permuted-basis index j -> original head dim (evens then odds)
    return np.concatenate([np.arange(0, HEAD_DIM, 2),
                           np.arange(1, HEAD_DIM, 2)])


def _host_tables():
    pos = np.arange(S, dtype=np.float32)
    freq = np.arange(0, HEAD_DIM, 2, dtype=np.float32) / HEAD_DIM
    inv_freq = 1.0 / (10000.0 ** freq)                  # [32]
    ang = np.outer(inv_freq, pos)                       # [32, S]
    cos1 = np.cos(ang)
    sin1 = np.sin(ang)
    # pi-basis per-head tables [64, S]: rows 0..31 evens, 32..63 odds
    cosh = np.concatenate([cos1, cos1], axis=0)
    sinh = np.concatenate([-sin1, sin1], axis=0)        # sign folded in
    cosT = np.tile(cosh, (2, 1))                        # [128, S] head pair
    sinT = np.tile(sinh, (2, 1))
    ident = np.eye(P, dtype=np.float32)
    # -240 on q<k entries of the diag block; exp(SCALE*(s-240)) ~ 0
    mneg = -240.0 * np.tril(np.ones((P, P), np.float32), -1)
    return (cosT.astype(BF16), sinT.astype(BF16),
            ident.astype(BF16), mneg.astype(BF16))


def kernel(x, Wq, bq, Wk, bk, Wv, bv, Wo, bo):
    from concourse.bass_utils import run_bass_kernel_spmd

    x = np.asarray(x, np.float32)
    Wq, Wk, Wv, Wo = (np.asarray(a, np.float32) for a in (Wq, Wk, Wv, Wo))
    bq, bk, bv, bo = (np.asarray(a, np.float32) for a in (bq, bk, bv, bo))

    # q/k biases interact with RoPE/softmax nonlinearly; the problem spec
    # fills all biases with zeros, so the device kernel drops them.
    assert np.abs(bq).max() == 0.0 and np.abs(bk).max() == 0.0, (
        "nonzero q/k biases not supported")

    if "nc" not in _CACHE:
        _CACHE["nc"] = _build_bass()
    nc = _CACHE["nc"]

    cosT, sinT, ident, mneg = _host_tables()
    consts = {"cosT": cosT, "sinT": sinT, "ident": ident, "mneg": mneg}

    # pi-basis permutation of q/k projection columns (per head)
    pi = _perm64()
    colperm = np.concatenate([h * HEAD_DIM + pi for h in range(N_HEADS)])
    Wq_p = Wq[:, colperm]
    Wk_p = Wk[:, colperm]

    xTs = [np.ascontiguousarray(x[b].T).astype(BF16) for b in range(B)]
    in_maps = []
    for c in range(N_CORES):
        b, g = c // HG, c % HG
        sl = slice(g * DL, (g + 1) * DL)
        in_maps.append({
            "xT": xTs[b],
            "wq": np.ascontiguousarray(Wq_p[:, sl]).astype(BF16),
            "wk": np.ascontiguousarray(Wk_p[:, sl]).astype(BF16),
            "wv": np.ascontiguousarray(Wv[:, sl]).astype(BF16),
            "wo": np.ascontiguousarray(Wo[sl, :]).astype(BF16),
            **consts,
        })

    res = run_bass_kernel_spmd(nc, in_maps, core_ids=list(range(N_CORES)))
    _CACHE["last_result"] = res
    out = np.empty((B, S, D_MODEL), np.float32)
    for b in range(B):
        out[b] = (res.results[HG * b]["o"].astype(np.float32) +
                  res.results[HG * b + 1]["o"].astype(np.float32))
    # v-bias passes through softmax exactly; bo is a plain add
    out += (bo + bv @ Wo).astype(np.float32)
    return out
